# revision 35
# baseline (speedup 1.0000x reference)
"""CBAM kernel for Trainium2, 8-core data-parallel (4 batches per core), bf16.

v2 design (vs fp32 v1 baseline at 192us):
- x converted to bf16 on host: halves HBM traffic (36us -> 18us in, same out)
  and doubles DVE tensor_tensor throughput (2x_1P mode, measured 1084ns per
  [128,1792] op). Gate math error budget validated on host: rel ~7.7e-3 vs
  2e-2 harness gate.
- channel-sum on PE via block-diag mask matmuls (bf16, N=512 groups).
- channel-max via DVE tensor_max chain riding the DMA-in.
- xg = x*cg in-place (DVE TT bf16 with broadcast-AP cg, stays 2x).
- smax/savg via DVE fold trees (bf16 2x) + final 1x tensor_reduce, at pair
  granularity to amortize op overhead.
- 7x7 conv as 14 PE matmuls with fp32r band matrices (full-rate fp32).
- out = xg*sg: per-n tensor_scalar (DVE 279ns) / ACT mul (600ns), split by
  chunk to balance engines; chunk-granular so each out-DMA waits on one
  engine.

Layout: per core [12544, 256] = 2 pairs x [128p, 49n, 256c]; flat row
r = 49*p + n within a pair puts batch = p//64 (3136 = 64*49): contiguous
per-partition DMA runs of 3.5KB.
"""

import numpy as np
import ml_dtypes
from contextlib import ExitStack

import concourse.bass as bass
import concourse.tile as tile
from concourse import mybir
from concourse._compat import with_exitstack
from concourse.tile import add_dep_helper

F32 = mybir.dt.float32
F32R = mybir.dt.float32r
BF16 = mybir.dt.bfloat16

C = 256
HID = 16
NPAIR = 2
NBLK = 49
CHUNK = 7
NCHUNK = NBLK // CHUNK
ROWS_PAIR = 128 * NBLK   # 6272
ROWS_CORE = NPAIR * ROWS_PAIR  # 12544
H = W = 56
SP = H * W  # 3136
N_CORES = 8

MU = mybir.AluOpType
AF = mybir.ActivationFunctionType

# final-mult chunk assignment: True = ACT, False = DVE (per pair, 7 chunks)
F_ON_ACT = {
    0: [True, True, True, True, True, True, False],
    1: [True, True, False, False, False, False, False],
}
PE_WARM_MMS = 28


def _ap(handle_ap, offset_elems, dims):
    base = handle_ap[tuple([slice(None)] * len(handle_ap.shape))]
    return bass.AP(tensor=base.tensor, offset=base.offset + offset_elems, ap=dims)


@with_exitstack
def _emit(ctx: ExitStack, tc: tile.TileContext):
    nc = tc.nc

    x_d = nc.dram_tensor("x", [ROWS_CORE, C], BF16, kind="ExternalInput")
    w1h_d = nc.dram_tensor("w1h", [128, 2, HID], F32, kind="ExternalInput")
    w1sh_d = nc.dram_tensor("w1sh", [128, 2, HID], F32, kind="ExternalInput")
    w2h_d = nc.dram_tensor("w2h", [HID, 2, 128], F32, kind="ExternalInput")
    b1c_d = nc.dram_tensor("b1c", [HID, 1], F32, kind="ExternalInput")
    b2t_d = nc.dram_tensor("b2t", [128, 2], F32, kind="ExternalInput")
    bands_d = nc.dram_tensor("bands", [H, 14, H], F32, kind="ExternalInput")
    identf_d = nc.dram_tensor("identf", [128, 128], F32, kind="ExternalInput")
    identb_d = nc.dram_tensor("identb", [128, 128], BF16, kind="ExternalInput")
    mask2_d = nc.dram_tensor("mask2", [128, 2], BF16, kind="ExternalInput")
    mask2t_d = nc.dram_tensor("mask2t", [2, 128], F32, kind="ExternalInput")
    convb_d = nc.dram_tensor("convb", [H, 1], F32, kind="ExternalInput")
    out_d = nc.dram_tensor("out", [ROWS_CORE, C], BF16, kind="ExternalOutput")

    xv = x_d[:, :].rearrange("(q p n) c -> q p n c", q=NPAIR, p=128)
    ov = out_d[:, :].rearrange("(q p n) c -> q p n c", q=NPAIR, p=128)

    constp = ctx.enter_context(tc.tile_pool(name="const", bufs=1))
    bigp = ctx.enter_context(tc.tile_pool(name="big", bufs=1))
    workp = ctx.enter_context(tc.tile_pool(name="work", bufs=1))
    psp1 = ctx.enter_context(tc.tile_pool(name="ps1", bufs=1, space="PSUM"))
    psp2 = ctx.enter_context(tc.tile_pool(name="ps2", bufs=2, space="PSUM"))

    # consts go on the scalar HWDGE ring so the sync ring starts streaming
    # x chunks immediately (per-lane depth-1 rings serialize completions)
    def const_load(name, shape, dtype, dram):
        t = constp.tile(shape, dtype, tag=name)
        nc.scalar.dma_start(t[tuple([slice(None)] * len(shape))], dram)
        return t

    # load order: earliest-needed consts first (scalar-ring DMAs serialize)
    mask2 = const_load("mask2", [128, 2], BF16, mask2_d[:, :])
    identb = const_load("identb", [128, 128], BF16, identb_d[:, :])
    b1c = const_load("b1c", [HID, 1], F32, b1c_d[:, :])
    b2t = const_load("b2t", [128, 2], F32, b2t_d[:, :])
    w1h = const_load("w1h", [128, 2, HID], F32, w1h_d[:, :, :])
    w1sh = const_load("w1sh", [128, 2, HID], F32, w1sh_d[:, :, :])
    w2h = const_load("w2h", [HID, 2, 128], F32, w2h_d[:, :, :])
    identf = const_load("identf", [128, 128], F32, identf_d[:, :])
    mask2t = const_load("mask2t", [2, 128], F32, mask2t_d[:, :])
    convb = const_load("convb", [H, 1], F32, convb_d[:, :])
    bands = const_load("bands", [H, 14, H], F32, bands_d[:, :, :])

    identfb, identbb = identf, identb
    w1hb, w1shb, w2hb, mask2tb = w1h, w1sh, w2h, mask2t

    bandsb = constp.tile([H, 14, H], F32R, tag="bandsb")

    # ACT sigmoid table preload (off critical path)
    warm = workp.tile([128, 8], F32, tag="warm")
    nc.vector.memset(warm[:, :], 0.0)
    nc.scalar.activation(out=warm[:, 0:8], in_=warm[:, 0:8], func=AF.Sigmoid,
                         bias=0.0, scale=1.0)

    # PE HAM warm-up: keep PE busy from t~8us so chsum matmuls run at 2.4GHz.
    # Source tile comes from a DVE memset, not a DMA, so this starts at once.
    warm_pe = workp.tile([128, 128], BF16, tag="warmpe")
    nc.vector.memset(warm_pe[:, :], 0.0)
    warm_ps = psp1.tile([128, 16], F32, tag="mlp0")
    for _ in range(PE_WARM_MMS):
        nc.tensor.matmul(warm_ps[:, :], lhsT=warm_pe[:, :],
                         rhs=warm_pe[:, 0:16], start=True, stop=True)

    def load(q):
        """Issue pair q's DMA-in chunks."""
        X = bigp.tile([128, NBLK, C], BF16, tag=f"x{q}")
        for k in range(NCHUNK):
            nc.sync.dma_start(
                X[:, k * CHUNK:(k + 1) * CHUNK, :],
                xv[q, :, k * CHUNK:(k + 1) * CHUNK, :],
            )
        return X

    def stats(q, X):
        """channel-max folds on DVE + channel-sums on PE."""
        aw = workp.tile([128, CHUNK, C], BF16, tag=f"aw{q}")
        chs = psp2.tile([2, 512], F32, tag="chs")
        for k in range(NCHUNK):
            blk = X[:, k * CHUNK:(k + 1) * CHUNK, :]
            if k == 0:
                nc.vector.tensor_copy(aw[:], blk)
            else:
                nc.vector.tensor_max(aw[:], aw[:], blk)
        # channel sums: 24 pair-groups of N=512 + final single N=256
        for g in range(24):
            nc.tensor.matmul(
                chs[:, :], lhsT=mask2[:, :],
                rhs=X[:, 2 * g:2 * g + 2, :].rearrange("p a b -> p (a b)"),
                start=(g == 0), stop=False,
            )
        nc.tensor.matmul(chs[:, 0:256], lhsT=mask2[:, :], rhs=X[:, 48, :],
                         start=False, stop=True)
        # fold aw -> acc [128, 256]
        nc.vector.tensor_max(aw[:, 0:3, :], aw[:, 0:3, :], aw[:, 3:6, :])
        nc.vector.tensor_max(aw[:, 0, :], aw[:, 0, :], aw[:, 1, :])
        nc.vector.tensor_max(aw[:, 0, :], aw[:, 0, :], aw[:, 2, :])
        nc.vector.tensor_max(aw[:, 0, :], aw[:, 0, :], aw[:, 6, :])
        return aw[:, 0, :], chs

    def mlp(q, acc, chs):
        """channel gate from stats; returns cgb_bf [128, 256] bf16.

        Hop-minimized: copies/relu/sigmoid run on ACT (idle at this point,
        PSUM sources), the j-group sum and the avg+max add are folded into
        PE transpose accumulation groups. DVE only does the two max TRs."""
        statsT = workp.tile([128, 2, 2, 2], F32, tag=f"stats{q}")
        sum2 = workp.tile([2, 2, 256], F32, tag=f"sum{q}")
        nc.scalar.copy(sum2[:, :, :],
                       chs[:, :].rearrange("p (a b) -> p a b", a=2))
        mlp_ps = psp1.tile([128, 16], F32, tag=f"mlp{q}")
        # one PSUM bank per pair holds tp (bf16), tpr and cgb (f32 views)
        aux = psp1.tile([128, 1024], BF16, tag=f"aux{q}")
        for h2 in range(2):
            tp = aux[:, 0:128]
            nc.tensor.transpose(tp, acc[:, h2 * 128:(h2 + 1) * 128],
                                identbb[:])
            nc.vector.tensor_reduce(
                out=statsT[:, h2, 1, :],
                in_=tp.rearrange("c (b p) -> c b p", b=2),
                axis=mybir.AxisListType.X, op=MU.max,
            )
            # avg stats: transpose the two 512-group halves, accumulated
            for j in range(2):
                nc.tensor.matmul(
                    mlp_ps[:, 2 * h2:2 * h2 + 2],
                    lhsT=sum2[:, j, h2 * 128:(h2 + 1) * 128],
                    rhs=identfb[0:2, 0:2],
                    is_transpose=True, start=(j == 0), stop=(j == 1),
                )
            nc.scalar.copy(
                statsT[:, h2, 0, :], mlp_ps[:, 2 * h2:2 * h2 + 2]
            )
        for stat in range(2):
            w1x = w1shb if stat == 0 else w1hb
            for h2 in range(2):
                nc.tensor.matmul(
                    mlp_ps[0:HID, 4 + 2 * stat:6 + 2 * stat],
                    lhsT=w1x[:, h2, :], rhs=statsT[:, h2, stat, :],
                    start=(h2 == 0), stop=(h2 == 1),
                )
        h_sb = workp.tile([HID, 2, 2], F32, tag=f"hsb{q}")
        nc.scalar.activation(
            out=h_sb[:],
            in_=mlp_ps[0:HID, 4:8].rearrange("p (s b) -> p s b", s=2),
            func=AF.Relu, bias=b1c[:], scale=1.0,
        )
        sigT = workp.tile([128, 2, 4], F32, tag=f"sig{q}")
        for h2 in range(2):
            cgp = mlp_ps[:, 8 + 4 * h2:12 + 4 * h2]
            nc.tensor.matmul(cgp, lhsT=w2hb[:, h2, :], rhs=h_sb[:, :, :],
                             start=True, stop=True)
            nc.scalar.activation(
                out=sigT[:, h2, :], in_=cgp, func=AF.Sigmoid,
                bias=b2t[:, h2:h2 + 1], scale=1.0,
            )
        cgr = workp.tile([2, 2, 128], F32, tag=f"cgr{q}")
        cgb_ps = aux[:, 512:1024].bitcast(F32)
        for h2 in range(2):
            tpr = aux[0:2, 256:512].bitcast(F32)
            # avg + max sigmoid outputs added via transpose accumulation
            for part in range(2):
                nc.tensor.matmul(
                    tpr, lhsT=sigT[:, h2, 2 * part:2 * part + 2],
                    rhs=identfb[:, :],
                    is_transpose=True, start=(part == 0), stop=(part == 1),
                )
            nc.scalar.copy(cgr[:, h2, :], tpr)
            nc.tensor.matmul(
                cgb_ps[:, h2 * 128:(h2 + 1) * 128],
                lhsT=mask2tb[:], rhs=cgr[:, h2, :],
                start=True, stop=True,
            )
        cgb = workp.tile([128, C], BF16, tag=f"cgb{q}")
        nc.scalar.copy(cgb[:], cgb_ps[:])
        return cgb

    def gate_mult(q, X, cgb):
        """xg = x * cg in place, chunked (bf16 TT 2x with broadcast AP)."""
        cgb_rep = bass.AP(tensor=cgb.tensor, offset=cgb.offset,
                          ap=[cgb.ap[0], [0, CHUNK], cgb.ap[1]])
        for k in range(NCHUNK):
            blk = X[:, k * CHUNK:(k + 1) * CHUNK, :]
            nc.vector.tensor_tensor(out=blk, in0=blk, in1=cgb_rep, op=MU.mult)

    def spatial_stats(q, X):
        """smax/savg fold trees at pair granularity.
        ssb layout [128, 2(stat), 7, 8]: 49 values as 7 stride-8 rows of 7 so
        the SBUF->SBUF shuffle DMA keeps a non-collapsible inner dim of 7."""
        fb = workp.tile([128, NBLK, 128], BF16, tag=f"fb{q}")
        ssb = workp.tile([128, 2, CHUNK, 8], F32, tag=f"ssb{q}")
        for stat, op in ((1, MU.max), (0, MU.add)):
            nc.vector.tensor_tensor(out=fb[:, :, :], in0=X[:, :, 0:128],
                                    in1=X[:, :, 128:256], op=op)
            nc.vector.tensor_tensor(out=fb[:, :, 0:64], in0=fb[:, :, 0:64],
                                    in1=fb[:, :, 64:128], op=op)
            nc.vector.tensor_tensor(out=fb[:, :, 0:32], in0=fb[:, :, 0:32],
                                    in1=fb[:, :, 32:64], op=op)
            nc.vector.tensor_reduce(out=ssb[:, stat, :, 0:7],
                                    in_=fb[:, :, 0:32],
                                    axis=mybir.AxisListType.X, op=op)
        return ssb

    def conv(q, ssb):
        """7x7x2->1 conv: direct SBUF->SBUF reshuffles (no DRAM bounce),
        f32r band matmuls, sigmoid, direct gather back."""
        s_sb = workp.tile([H, 2, 2, 62], F32, tag=f"ssb2{q}")
        nc.vector.memset(s_sb[:], 0.0)
        # (p', n) walk == (h, w) walk per (ic, b): flat s = 49 p' + n = 56 h + w
        for ic in range(2):
            for b in range(2):
                nc.sync.dma_start(
                    s_sb[0:H, ic, b, 3:3 + W],
                    ssb[64 * b:64 * (b + 1), ic, :, 0:7],
                )
        s_sb2 = workp.tile([H, 2, 2, 62], F32R, tag=f"ssb3{q}")
        nc.vector.tensor_copy(s_sb2[:], s_sb[:])
        conv_ps = psp2.tile([H, 2, W], F32, tag="conv")
        for ic in range(2):
            for dw in range(7):
                j = ic * 7 + dw
                nc.tensor.matmul(
                    conv_ps[:], lhsT=bandsb[:, j, :],
                    rhs=s_sb2[:, ic, :, dw:dw + W],
                    start=(j == 0), stop=(j == 13),
                )
        sg_hw = workp.tile([H, 2, W], F32, tag=f"sghw{q}")
        nc.scalar.activation(out=sg_hw[:], in_=conv_ps[:], func=AF.Sigmoid,
                             bias=convb[:], scale=1.0)
        sg = workp.tile([128, CHUNK, 8], F32, tag=f"sg{q}")
        for b in range(2):
            nc.sync.dma_start(
                sg[64 * b:64 * (b + 1), :, 0:7],
                sg_hw[0:H, b, :],
            )
        return sg

    def finalize(q, X, sg, chunks, on_act):
        """out = xg * sg for the selected chunks, then DMA-out each chunk."""
        for k in chunks:
            for n in range(k * CHUNK, (k + 1) * CHUNK):
                sgn = sg[:, n // 7, (n % 7):(n % 7) + 1]
                if on_act:
                    nc.scalar.mul(X[:, n, :], X[:, n, :], mul=sgn)
                else:
                    nc.vector.tensor_scalar_mul(X[:, n, :], X[:, n, :], sgn)
            nc.sync.dma_start(
                ov[q, :, k * CHUNK:(k + 1) * CHUNK, :],
                X[:, k * CHUNK:(k + 1) * CHUNK, :],
            )

    def act_chunks(q):
        return [k for k in range(NCHUNK) if F_ON_ACT[q][k]]

    def dve_chunks(q):
        return [k for k in range(NCHUNK) if not F_ON_ACT[q][k]]

    # pipeline-ordered emission. Both loads are issued first (SP ring);
    # pair 1's DVE stats come AFTER C0 so the late-arriving X1 chunks don't
    # block C0 in the in-order DVE queue. F's DVE chunks come last.
    X0 = load(0)
    X1 = load(1)
    acc0, chs0 = stats(0, X0)
    cgb0 = mlp(0, acc0, chs0)
    gate_mult(0, X0, cgb0)
    # f32r producer for the conv band matmuls, emitted here so the DVE copy
    # doesn't head-block the queue waiting for the bands const DMA
    nc.vector.tensor_copy(bandsb[:, :, :], bands[:, :, :])
    acc1, chs1 = stats(1, X1)
    cgb1 = mlp(1, acc1, chs1)
    ssb0 = spatial_stats(0, X0)
    sg0 = conv(0, ssb0)
    finalize(0, X0, sg0, act_chunks(0), on_act=True)
    gate_mult(1, X1, cgb1)
    ssb1 = spatial_stats(1, X1)
    sg1 = conv(1, ssb1)
    finalize(0, X0, sg0, dve_chunks(0), on_act=False)
    finalize(1, X1, sg1, dve_chunks(1), on_act=False)
    finalize(1, X1, sg1, act_chunks(1), on_act=True)


def _split_evsem_clears(nc):
    """Walrus rejects EVENT_SEMAPHORE_RANGE_CLEAR over wide sem ranges;
    split into clears of <=3 sems."""
    for f in nc.m.functions:
        for blk in f.blocks:
            il = blk.instructions
            for i in range(len(il)):
                inst = il[i]
                if type(inst).__name__ != 'InstISA':
                    continue
                d = inst.ant_dict
                if d is None or 'range_first' not in d or 'range_last' not in d:
                    continue
                first, last = d['range_first'], d['range_last']
                if last - first + 1 <= 3:
                    continue
                si = inst.sync_info
                import copy
                reps = []
                a = first
                while a <= last:
                    b = min(a + 2, last)
                    cl = copy.deepcopy(inst)
                    cl.name = f"I-ws{nc.next_id()}"
                    cd = cl.ant_dict
                    cd['range_first'] = a
                    cd['range_last'] = b
                    reps.append(cl)
                    a = b + 1
                reps[0].sync_info = si
                il[i] = reps[0]
                for j, r in enumerate(reps[1:]):
                    il.insert(i + 1 + j, r)
                break


def _split_waits(nc):
    """Walrus accepts at most ONE sync wait per engine instruction; split
    surplus waits onto injected drain carriers (same engine, order kept)."""
    import copy

    proto = {}
    for f in nc.m.functions:
        for blk in f.blocks:
            for inst in blk.instructions:
                if type(inst).__name__ == 'InstDrain' and inst.engine not in proto:
                    proto[inst.engine] = inst
    for f in nc.m.functions:
        for blk in f.blocks:
            il = blk.instructions
            i = 0
            while i < len(il):
                inst = il[i]
                si = inst.sync_info
                if si is None or len(si.on_wait) <= 1:
                    i += 1
                    continue
                waits = list(si.on_wait)
                eng = inst.engine
                for w in waits[:-1]:
                    nop = copy.deepcopy(proto[eng])
                    nop.name = f"I-ws{nc.next_id()}"
                    nop.sync_info = type(si)(on_wait=[w], on_update=[])
                    il.insert(i, nop)
                    i += 1
                inst.sync_info = type(si)(
                    on_wait=[waits[-1]], on_update=list(si.on_update)
                )
                i += 1


_NC = {}


def _get_nc(split=True):
    if split not in _NC:
        nc = bass.Bass()
        with tile.TileContext(nc) as tc:
            _emit(tc)
        if split:
            _split_waits(nc)
            _split_evsem_clears(nc)
        _NC[split] = nc
    return _NC[split]


def _host_inputs(w1, b1, w2, b2, conv_w, conv_b):
    w1 = np.asarray(w1, np.float32)
    w2 = np.asarray(w2, np.float32)
    w1h = np.ascontiguousarray(w1.reshape(2, 128, HID).transpose(1, 0, 2))
    w1sh = np.ascontiguousarray(w1h / float(SP))
    w2h = np.ascontiguousarray(np.asarray(w2, np.float32).reshape(HID, 2, 128))
    b1c = np.ascontiguousarray(np.asarray(b1, np.float32).reshape(HID, 1))
    b2t = np.ascontiguousarray(np.asarray(b2, np.float32).reshape(2, 128).T)
    cw = np.asarray(conv_w, np.float32).reshape(7, 7, 2)
    bands = np.zeros((H, 14, H), np.float32)
    for ic in range(2):
        for dw in range(7):
            for dh in range(7):
                d = dh - 3
                v = cw[dh, dw, ic]
                if ic == 0:
                    v = v / float(C)  # fold 1/C of s_avg into avg bands
                if d >= 0:
                    idx = np.arange(0, H - d)
                    bands[idx + d, ic * 7 + dw, idx] = v
                else:
                    idx = np.arange(-d, H)
                    bands[idx + d, ic * 7 + dw, idx] = v
    identf = np.eye(128, dtype=np.float32)
    identb = np.eye(128, dtype=np.float32).astype(ml_dtypes.bfloat16)
    mask2 = np.zeros((128, 2), np.float32)
    mask2[0:64, 0] = 1.0
    mask2[64:128, 1] = 1.0
    mask2b = mask2.astype(ml_dtypes.bfloat16)
    mask2t = np.ascontiguousarray(mask2.T)
    convb = np.full((H, 1), np.asarray(conv_b, np.float32).reshape(-1)[0],
                    np.float32)
    return dict(w1h=w1h, w1sh=w1sh, w2h=w2h, b1c=b1c, b2t=b2t,
                bands=bands, identf=identf, identb=identb, mask2=mask2b,
                mask2t=mask2t, convb=convb)


def kernel(x, w1, b1, w2, b2, conv_w, conv_b, _trace=False):
    from concourse.bass_utils import run_bass_kernel_spmd

    nc = _get_nc()
    consts = _host_inputs(w1, b1, w2, b2, conv_w, conv_b)
    xb = np.asarray(x, np.float32).astype(ml_dtypes.bfloat16)
    xs = np.ascontiguousarray(xb).reshape(8, ROWS_CORE, C)
    in_maps = [dict(consts, x=xs[i]) for i in range(N_CORES)]
    res = run_bass_kernel_spmd(nc, in_maps, core_ids=list(range(N_CORES)),
                               trace=_trace)
    out = np.stack([np.asarray(r["out"]) for r in res.results])
    out = out.astype(np.float32).reshape(32, H, W, C)
    if _trace:
        kernel.last_results = res
    return out


# revision 37
# speedup vs baseline: 1.0225x; 1.0225x over previous
"""CBAM kernel for Trainium2, 8-core data-parallel (4 batches per core), bf16.

v2 design (vs fp32 v1 baseline at 192us):
- x converted to bf16 on host: halves HBM traffic (36us -> 18us in, same out)
  and doubles DVE tensor_tensor throughput (2x_1P mode, measured 1084ns per
  [128,1792] op). Gate math error budget validated on host: rel ~7.7e-3 vs
  2e-2 harness gate.
- channel-sum on PE via block-diag mask matmuls (bf16, N=512 groups).
- channel-max via DVE tensor_max chain riding the DMA-in.
- xg = x*cg in-place (DVE TT bf16 with broadcast-AP cg, stays 2x).
- smax/savg via DVE fold trees (bf16 2x) + final 1x tensor_reduce, at pair
  granularity to amortize op overhead.
- 7x7 conv as 14 PE matmuls with fp32r band matrices (full-rate fp32).
- out = xg*sg: per-n tensor_scalar (DVE 279ns) / ACT mul (600ns), split by
  chunk to balance engines; chunk-granular so each out-DMA waits on one
  engine.

Layout: per core [12544, 256] = 2 pairs x [128p, 49n, 256c]; flat row
r = 49*p + n within a pair puts batch = p//64 (3136 = 64*49): contiguous
per-partition DMA runs of 3.5KB.
"""

import numpy as np
import ml_dtypes
from contextlib import ExitStack

import concourse.bass as bass
import concourse.tile as tile
from concourse import mybir
from concourse._compat import with_exitstack
from concourse.tile import add_dep_helper

F32 = mybir.dt.float32
F32R = mybir.dt.float32r
BF16 = mybir.dt.bfloat16

C = 256
HID = 16
NPAIR = 2
NBLK = 49
CHUNK = 7
NCHUNK = NBLK // CHUNK
ROWS_PAIR = 128 * NBLK   # 6272
ROWS_CORE = NPAIR * ROWS_PAIR  # 12544
H = W = 56
SP = H * W  # 3136
N_CORES = 8

MU = mybir.AluOpType
AF = mybir.ActivationFunctionType

# final-mult chunk assignment: True = ACT, False = DVE (per pair, 7 chunks)
F_ON_ACT = {
    0: [True, True, True, True, True, False, False],
    1: [True, True, False, False, False, False, False],
}
PE_WARM_MMS = 28


def _ap(handle_ap, offset_elems, dims):
    base = handle_ap[tuple([slice(None)] * len(handle_ap.shape))]
    return bass.AP(tensor=base.tensor, offset=base.offset + offset_elems, ap=dims)


@with_exitstack
def _emit(ctx: ExitStack, tc: tile.TileContext):
    nc = tc.nc

    x_d = nc.dram_tensor("x", [ROWS_CORE, C], BF16, kind="ExternalInput")
    w1h_d = nc.dram_tensor("w1h", [128, 2, HID], F32, kind="ExternalInput")
    w1sh_d = nc.dram_tensor("w1sh", [128, 2, HID], F32, kind="ExternalInput")
    w2h_d = nc.dram_tensor("w2h", [HID, 2, 128], F32, kind="ExternalInput")
    b1c_d = nc.dram_tensor("b1c", [HID, 1], F32, kind="ExternalInput")
    b2t_d = nc.dram_tensor("b2t", [128, 2], F32, kind="ExternalInput")
    bands_d = nc.dram_tensor("bands", [H, 14, H], F32, kind="ExternalInput")
    identf_d = nc.dram_tensor("identf", [128, 128], F32, kind="ExternalInput")
    identb_d = nc.dram_tensor("identb", [128, 128], BF16, kind="ExternalInput")
    mask2_d = nc.dram_tensor("mask2", [128, 2], BF16, kind="ExternalInput")
    mask2t_d = nc.dram_tensor("mask2t", [2, 128], F32, kind="ExternalInput")
    convb_d = nc.dram_tensor("convb", [H, 1], F32, kind="ExternalInput")
    out_d = nc.dram_tensor("out", [ROWS_CORE, C], BF16, kind="ExternalOutput")

    xv = x_d[:, :].rearrange("(q p n) c -> q p n c", q=NPAIR, p=128)
    ov = out_d[:, :].rearrange("(q p n) c -> q p n c", q=NPAIR, p=128)

    constp = ctx.enter_context(tc.tile_pool(name="const", bufs=1))
    bigp = ctx.enter_context(tc.tile_pool(name="big", bufs=1))
    workp = ctx.enter_context(tc.tile_pool(name="work", bufs=1))
    psp1 = ctx.enter_context(tc.tile_pool(name="ps1", bufs=1, space="PSUM"))
    psp2 = ctx.enter_context(tc.tile_pool(name="ps2", bufs=2, space="PSUM"))

    # consts go on the scalar HWDGE ring so the sync ring starts streaming
    # x chunks immediately (per-lane depth-1 rings serialize completions)
    def const_load(name, shape, dtype, dram):
        t = constp.tile(shape, dtype, tag=name)
        nc.scalar.dma_start(t[tuple([slice(None)] * len(shape))], dram)
        return t

    # load order: earliest-needed consts first (scalar-ring DMAs serialize)
    mask2 = const_load("mask2", [128, 2], BF16, mask2_d[:, :])
    identb = const_load("identb", [128, 128], BF16, identb_d[:, :])
    b1c = const_load("b1c", [HID, 1], F32, b1c_d[:, :])
    b2t = const_load("b2t", [128, 2], F32, b2t_d[:, :])
    w1h = const_load("w1h", [128, 2, HID], F32, w1h_d[:, :, :])
    w1sh = const_load("w1sh", [128, 2, HID], F32, w1sh_d[:, :, :])
    w2h = const_load("w2h", [HID, 2, 128], F32, w2h_d[:, :, :])
    identf = const_load("identf", [128, 128], F32, identf_d[:, :])
    mask2t = const_load("mask2t", [2, 128], F32, mask2t_d[:, :])
    convb = const_load("convb", [H, 1], F32, convb_d[:, :])
    bands = const_load("bands", [H, 14, H], F32, bands_d[:, :, :])

    identfb, identbb = identf, identb
    w1hb, w1shb, w2hb, mask2tb = w1h, w1sh, w2h, mask2t

    bandsb = constp.tile([H, 14, H], F32R, tag="bandsb")

    # ACT sigmoid table preload (off critical path)
    warm = workp.tile([128, 8], F32, tag="warm")
    nc.vector.memset(warm[:, :], 0.0)
    nc.scalar.activation(out=warm[:, 0:8], in_=warm[:, 0:8], func=AF.Sigmoid,
                         bias=0.0, scale=1.0)

    # PE HAM warm-up: keep PE busy from t~8us so chsum matmuls run at 2.4GHz.
    # Source tile comes from a DVE memset, not a DMA, so this starts at once.
    warm_pe = workp.tile([128, 128], BF16, tag="warmpe")
    nc.vector.memset(warm_pe[:, :], 0.0)
    warm_ps = psp1.tile([128, 16], F32, tag="mlp0")
    for _ in range(PE_WARM_MMS):
        nc.tensor.matmul(warm_ps[:, :], lhsT=warm_pe[:, :],
                         rhs=warm_pe[:, 0:16], start=True, stop=True)

    def load(q):
        """Issue pair q's DMA-in chunks."""
        X = bigp.tile([128, NBLK, C], BF16, tag=f"x{q}")
        for k in range(NCHUNK):
            nc.sync.dma_start(
                X[:, k * CHUNK:(k + 1) * CHUNK, :],
                xv[q, :, k * CHUNK:(k + 1) * CHUNK, :],
            )
        return X

    def stats(q, X):
        """channel-max folds on DVE + channel-sums on PE."""
        aw = workp.tile([128, CHUNK, C], BF16, tag=f"aw{q}")
        chs = psp2.tile([2, 512], F32, tag="chs")
        for k in range(NCHUNK):
            blk = X[:, k * CHUNK:(k + 1) * CHUNK, :]
            if k == 0:
                nc.vector.tensor_copy(aw[:], blk)
            else:
                nc.vector.tensor_max(aw[:], aw[:], blk)
        # channel sums: 24 pair-groups of N=512 + final single N=256
        for g in range(24):
            nc.tensor.matmul(
                chs[:, :], lhsT=mask2[:, :],
                rhs=X[:, 2 * g:2 * g + 2, :].rearrange("p a b -> p (a b)"),
                start=(g == 0), stop=False,
            )
        nc.tensor.matmul(chs[:, 0:256], lhsT=mask2[:, :], rhs=X[:, 48, :],
                         start=False, stop=True)
        # fold aw -> acc [128, 256]
        nc.vector.tensor_max(aw[:, 0:3, :], aw[:, 0:3, :], aw[:, 3:6, :])
        nc.vector.tensor_max(aw[:, 0, :], aw[:, 0, :], aw[:, 1, :])
        nc.vector.tensor_max(aw[:, 0, :], aw[:, 0, :], aw[:, 2, :])
        nc.vector.tensor_max(aw[:, 0, :], aw[:, 0, :], aw[:, 6, :])
        return aw[:, 0, :], chs

    def mlp(q, acc, chs):
        """channel gate from stats; returns cgb_bf [128, 256] bf16.

        Hop-minimized: copies/relu/sigmoid run on ACT (idle at this point,
        PSUM sources), the j-group sum and the avg+max add are folded into
        PE transpose accumulation groups. DVE only does the two max TRs."""
        statsT = workp.tile([128, 2, 2, 2], F32, tag=f"stats{q}")
        sum2 = workp.tile([2, 2, 256], F32, tag=f"sum{q}")
        nc.scalar.copy(sum2[:, :, :],
                       chs[:, :].rearrange("p (a b) -> p a b", a=2))
        mlp_ps = psp1.tile([128, 16], F32, tag=f"mlp{q}")
        # one PSUM bank per pair holds tp (bf16), tpr and cgb (f32 views)
        aux = psp1.tile([128, 1024], BF16, tag=f"aux{q}")
        for h2 in range(2):
            tp = aux[:, 0:128]
            nc.tensor.transpose(tp, acc[:, h2 * 128:(h2 + 1) * 128],
                                identbb[:])
            nc.vector.tensor_reduce(
                out=statsT[:, h2, 1, :],
                in_=tp.rearrange("c (b p) -> c b p", b=2),
                axis=mybir.AxisListType.X, op=MU.max,
            )
            # avg stats: transpose the two 512-group halves, accumulated
            for j in range(2):
                nc.tensor.matmul(
                    mlp_ps[:, 2 * h2:2 * h2 + 2],
                    lhsT=sum2[:, j, h2 * 128:(h2 + 1) * 128],
                    rhs=identfb[0:2, 0:2],
                    is_transpose=True, start=(j == 0), stop=(j == 1),
                )
            nc.scalar.copy(
                statsT[:, h2, 0, :], mlp_ps[:, 2 * h2:2 * h2 + 2]
            )
        for stat in range(2):
            w1x = w1shb if stat == 0 else w1hb
            for h2 in range(2):
                nc.tensor.matmul(
                    mlp_ps[0:HID, 4 + 2 * stat:6 + 2 * stat],
                    lhsT=w1x[:, h2, :], rhs=statsT[:, h2, stat, :],
                    start=(h2 == 0), stop=(h2 == 1),
                )
        h_sb = workp.tile([HID, 2, 2], F32, tag=f"hsb{q}")
        nc.scalar.activation(
            out=h_sb[:],
            in_=mlp_ps[0:HID, 4:8].rearrange("p (s b) -> p s b", s=2),
            func=AF.Relu, bias=b1c[:], scale=1.0,
        )
        sigT = workp.tile([128, 2, 4], F32, tag=f"sig{q}")
        for h2 in range(2):
            cgp = mlp_ps[:, 8 + 4 * h2:12 + 4 * h2]
            nc.tensor.matmul(cgp, lhsT=w2hb[:, h2, :], rhs=h_sb[:, :, :],
                             start=True, stop=True)
            nc.scalar.activation(
                out=sigT[:, h2, :], in_=cgp, func=AF.Sigmoid,
                bias=b2t[:, h2:h2 + 1], scale=1.0,
            )
        cgr = workp.tile([2, 2, 128], F32, tag=f"cgr{q}")
        cgb_ps = aux[:, 512:1024].bitcast(F32)
        for h2 in range(2):
            tpr = aux[0:2, 256:512].bitcast(F32)
            # avg + max sigmoid outputs added via transpose accumulation
            for part in range(2):
                nc.tensor.matmul(
                    tpr, lhsT=sigT[:, h2, 2 * part:2 * part + 2],
                    rhs=identfb[:, :],
                    is_transpose=True, start=(part == 0), stop=(part == 1),
                )
            nc.scalar.copy(cgr[:, h2, :], tpr)
            nc.tensor.matmul(
                cgb_ps[:, h2 * 128:(h2 + 1) * 128],
                lhsT=mask2tb[:], rhs=cgr[:, h2, :],
                start=True, stop=True,
            )
        cgb = workp.tile([128, C], BF16, tag=f"cgb{q}")
        nc.scalar.copy(cgb[:], cgb_ps[:])
        return cgb

    def gate_mult(q, X, cgb):
        """xg = x * cg in place, chunked (bf16 TT 2x with broadcast AP)."""
        cgb_rep = bass.AP(tensor=cgb.tensor, offset=cgb.offset,
                          ap=[cgb.ap[0], [0, CHUNK], cgb.ap[1]])
        for k in range(NCHUNK):
            blk = X[:, k * CHUNK:(k + 1) * CHUNK, :]
            nc.vector.tensor_tensor(out=blk, in0=blk, in1=cgb_rep, op=MU.mult)

    def spatial_stats(q, X):
        """smax/savg fold trees at pair granularity.
        ssb layout [128, 2(stat), 7, 8]: 49 values as 7 stride-8 rows of 7 so
        the SBUF->SBUF shuffle DMA keeps a non-collapsible inner dim of 7."""
        fb = workp.tile([128, NBLK, 128], BF16, tag=f"fb{q}")
        ssb = workp.tile([128, 2, CHUNK, 8], F32, tag=f"ssb{q}")
        for stat, op in ((1, MU.max), (0, MU.add)):
            nc.vector.tensor_tensor(out=fb[:, :, :], in0=X[:, :, 0:128],
                                    in1=X[:, :, 128:256], op=op)
            nc.vector.tensor_tensor(out=fb[:, :, 0:64], in0=fb[:, :, 0:64],
                                    in1=fb[:, :, 64:128], op=op)
            nc.vector.tensor_tensor(out=fb[:, :, 0:32], in0=fb[:, :, 0:32],
                                    in1=fb[:, :, 32:64], op=op)
            nc.vector.tensor_reduce(out=ssb[:, stat, :, 0:7],
                                    in_=fb[:, :, 0:32],
                                    axis=mybir.AxisListType.X, op=op)
        return ssb

    def conv(q, ssb):
        """7x7x2->1 conv: direct SBUF->SBUF reshuffles (no DRAM bounce),
        f32r band matmuls, sigmoid, direct gather back."""
        s_sb = workp.tile([H, 2, 2, 62], F32, tag=f"ssb2{q}")
        nc.vector.memset(s_sb[:], 0.0)
        # (p', n) walk == (h, w) walk per (ic, b): flat s = 49 p' + n = 56 h + w
        for ic in range(2):
            for b in range(2):
                nc.sync.dma_start(
                    s_sb[0:H, ic, b, 3:3 + W],
                    ssb[64 * b:64 * (b + 1), ic, :, 0:7],
                )
        s_sb2 = workp.tile([H, 2, 2, 62], F32R, tag=f"ssb3{q}")
        nc.vector.tensor_copy(s_sb2[:], s_sb[:])
        conv_ps = psp2.tile([H, 2, W], F32, tag="conv")
        for ic in range(2):
            for dw in range(7):
                j = ic * 7 + dw
                nc.tensor.matmul(
                    conv_ps[:], lhsT=bandsb[:, j, :],
                    rhs=s_sb2[:, ic, :, dw:dw + W],
                    start=(j == 0), stop=(j == 13),
                )
        sg_hw = workp.tile([H, 2, W], F32, tag=f"sghw{q}")
        nc.scalar.activation(out=sg_hw[:], in_=conv_ps[:], func=AF.Sigmoid,
                             bias=convb[:], scale=1.0)
        sg = workp.tile([128, CHUNK, 8], F32, tag=f"sg{q}")
        for b in range(2):
            nc.sync.dma_start(
                sg[64 * b:64 * (b + 1), :, 0:7],
                sg_hw[0:H, b, :],
            )
        return sg

    def finalize(q, X, sg, chunks, on_act):
        """out = xg * sg for the selected chunks, then DMA-out each chunk."""
        for k in chunks:
            for n in range(k * CHUNK, (k + 1) * CHUNK):
                sgn = sg[:, n // 7, (n % 7):(n % 7) + 1]
                if on_act:
                    nc.scalar.mul(X[:, n, :], X[:, n, :], mul=sgn)
                else:
                    nc.vector.tensor_scalar_mul(X[:, n, :], X[:, n, :], sgn)
            nc.sync.dma_start(
                ov[q, :, k * CHUNK:(k + 1) * CHUNK, :],
                X[:, k * CHUNK:(k + 1) * CHUNK, :],
            )

    def act_chunks(q):
        return [k for k in range(NCHUNK) if F_ON_ACT[q][k]]

    def dve_chunks(q):
        return [k for k in range(NCHUNK) if not F_ON_ACT[q][k]]

    # pipeline-ordered emission. Both loads are issued first (SP ring);
    # pair 1's DVE stats come AFTER C0 so the late-arriving X1 chunks don't
    # block C0 in the in-order DVE queue. F's DVE chunks come last.
    X0 = load(0)
    X1 = load(1)
    acc0, chs0 = stats(0, X0)
    cgb0 = mlp(0, acc0, chs0)
    acc1, chs1 = stats(1, X1)
    cgb1 = mlp(1, acc1, chs1)
    gate_mult(0, X0, cgb0)
    # f32r producer for the conv band matmuls, emitted here so the DVE copy
    # doesn't head-block the queue waiting for the bands const DMA
    nc.vector.tensor_copy(bandsb[:, :, :], bands[:, :, :])
    ssb0 = spatial_stats(0, X0)
    sg0 = conv(0, ssb0)
    finalize(0, X0, sg0, act_chunks(0), on_act=True)
    gate_mult(1, X1, cgb1)
    ssb1 = spatial_stats(1, X1)
    sg1 = conv(1, ssb1)
    finalize(0, X0, sg0, dve_chunks(0), on_act=False)
    finalize(1, X1, sg1, dve_chunks(1), on_act=False)
    finalize(1, X1, sg1, act_chunks(1), on_act=True)


def _split_evsem_clears(nc):
    """Walrus rejects EVENT_SEMAPHORE_RANGE_CLEAR over wide sem ranges;
    split into clears of <=3 sems."""
    for f in nc.m.functions:
        for blk in f.blocks:
            il = blk.instructions
            for i in range(len(il)):
                inst = il[i]
                if type(inst).__name__ != 'InstISA':
                    continue
                d = inst.ant_dict
                if d is None or 'range_first' not in d or 'range_last' not in d:
                    continue
                first, last = d['range_first'], d['range_last']
                if last - first + 1 <= 3:
                    continue
                si = inst.sync_info
                import copy
                reps = []
                a = first
                while a <= last:
                    b = min(a + 2, last)
                    cl = copy.deepcopy(inst)
                    cl.name = f"I-ws{nc.next_id()}"
                    cd = cl.ant_dict
                    cd['range_first'] = a
                    cd['range_last'] = b
                    reps.append(cl)
                    a = b + 1
                reps[0].sync_info = si
                il[i] = reps[0]
                for j, r in enumerate(reps[1:]):
                    il.insert(i + 1 + j, r)
                break


def _split_waits(nc):
    """Walrus accepts at most ONE sync wait per engine instruction; split
    surplus waits onto injected drain carriers (same engine, order kept)."""
    import copy

    proto = {}
    for f in nc.m.functions:
        for blk in f.blocks:
            for inst in blk.instructions:
                if type(inst).__name__ == 'InstDrain' and inst.engine not in proto:
                    proto[inst.engine] = inst
    for f in nc.m.functions:
        for blk in f.blocks:
            il = blk.instructions
            i = 0
            while i < len(il):
                inst = il[i]
                si = inst.sync_info
                if si is None or len(si.on_wait) <= 1:
                    i += 1
                    continue
                waits = list(si.on_wait)
                eng = inst.engine
                for w in waits[:-1]:
                    nop = copy.deepcopy(proto[eng])
                    nop.name = f"I-ws{nc.next_id()}"
                    nop.sync_info = type(si)(on_wait=[w], on_update=[])
                    il.insert(i, nop)
                    i += 1
                inst.sync_info = type(si)(
                    on_wait=[waits[-1]], on_update=list(si.on_update)
                )
                i += 1


_NC = {}


def _get_nc(split=True):
    if split not in _NC:
        nc = bass.Bass()
        with tile.TileContext(nc) as tc:
            _emit(tc)
        if split:
            _split_waits(nc)
            _split_evsem_clears(nc)
        _NC[split] = nc
    return _NC[split]


def _host_inputs(w1, b1, w2, b2, conv_w, conv_b):
    w1 = np.asarray(w1, np.float32)
    w2 = np.asarray(w2, np.float32)
    w1h = np.ascontiguousarray(w1.reshape(2, 128, HID).transpose(1, 0, 2))
    w1sh = np.ascontiguousarray(w1h / float(SP))
    w2h = np.ascontiguousarray(np.asarray(w2, np.float32).reshape(HID, 2, 128))
    b1c = np.ascontiguousarray(np.asarray(b1, np.float32).reshape(HID, 1))
    b2t = np.ascontiguousarray(np.asarray(b2, np.float32).reshape(2, 128).T)
    cw = np.asarray(conv_w, np.float32).reshape(7, 7, 2)
    bands = np.zeros((H, 14, H), np.float32)
    for ic in range(2):
        for dw in range(7):
            for dh in range(7):
                d = dh - 3
                v = cw[dh, dw, ic]
                if ic == 0:
                    v = v / float(C)  # fold 1/C of s_avg into avg bands
                if d >= 0:
                    idx = np.arange(0, H - d)
                    bands[idx + d, ic * 7 + dw, idx] = v
                else:
                    idx = np.arange(-d, H)
                    bands[idx + d, ic * 7 + dw, idx] = v
    identf = np.eye(128, dtype=np.float32)
    identb = np.eye(128, dtype=np.float32).astype(ml_dtypes.bfloat16)
    mask2 = np.zeros((128, 2), np.float32)
    mask2[0:64, 0] = 1.0
    mask2[64:128, 1] = 1.0
    mask2b = mask2.astype(ml_dtypes.bfloat16)
    mask2t = np.ascontiguousarray(mask2.T)
    convb = np.full((H, 1), np.asarray(conv_b, np.float32).reshape(-1)[0],
                    np.float32)
    return dict(w1h=w1h, w1sh=w1sh, w2h=w2h, b1c=b1c, b2t=b2t,
                bands=bands, identf=identf, identb=identb, mask2=mask2b,
                mask2t=mask2t, convb=convb)


def kernel(x, w1, b1, w2, b2, conv_w, conv_b, _trace=False):
    from concourse.bass_utils import run_bass_kernel_spmd

    nc = _get_nc()
    consts = _host_inputs(w1, b1, w2, b2, conv_w, conv_b)
    xb = np.asarray(x, np.float32).astype(ml_dtypes.bfloat16)
    xs = np.ascontiguousarray(xb).reshape(8, ROWS_CORE, C)
    in_maps = [dict(consts, x=xs[i]) for i in range(N_CORES)]
    res = run_bass_kernel_spmd(nc, in_maps, core_ids=list(range(N_CORES)),
                               trace=_trace)
    out = np.stack([np.asarray(r["out"]) for r in res.results])
    out = out.astype(np.float32).reshape(32, H, W, C)
    if _trace:
        kernel.last_results = res
    return out


# revision 39
# speedup vs baseline: 1.0609x; 1.0376x over previous
"""CBAM kernel for Trainium2, 8-core data-parallel (4 batches per core), bf16.

v2 design (vs fp32 v1 baseline at 192us):
- x converted to bf16 on host: halves HBM traffic (36us -> 18us in, same out)
  and doubles DVE tensor_tensor throughput (2x_1P mode, measured 1084ns per
  [128,1792] op). Gate math error budget validated on host: rel ~7.7e-3 vs
  2e-2 harness gate.
- channel-sum on PE via block-diag mask matmuls (bf16, N=512 groups).
- channel-max via DVE tensor_max chain riding the DMA-in.
- xg = x*cg in-place (DVE TT bf16 with broadcast-AP cg, stays 2x).
- smax/savg via DVE fold trees (bf16 2x) + final 1x tensor_reduce, at pair
  granularity to amortize op overhead.
- 7x7 conv as 14 PE matmuls with fp32r band matrices (full-rate fp32).
- out = xg*sg: per-n tensor_scalar (DVE 279ns) / ACT mul (600ns), split by
  chunk to balance engines; chunk-granular so each out-DMA waits on one
  engine.

Layout: per core [12544, 256] = 2 pairs x [128p, 49n, 256c]; flat row
r = 49*p + n within a pair puts batch = p//64 (3136 = 64*49): contiguous
per-partition DMA runs of 3.5KB.
"""

import numpy as np
import ml_dtypes
from contextlib import ExitStack

import concourse.bass as bass
import concourse.tile as tile
from concourse import mybir
from concourse._compat import with_exitstack
from concourse.tile import add_dep_helper

F32 = mybir.dt.float32
F32R = mybir.dt.float32r
BF16 = mybir.dt.bfloat16

C = 256
HID = 16
NPAIR = 2
NBLK = 49
CHUNK = 7
NCHUNK = NBLK // CHUNK
ROWS_PAIR = 128 * NBLK   # 6272
ROWS_CORE = NPAIR * ROWS_PAIR  # 12544
H = W = 56
SP = H * W  # 3136
N_CORES = 8

MU = mybir.AluOpType
AF = mybir.ActivationFunctionType

# final-mult chunk assignment: True = ACT, False = DVE (per pair, 7 chunks)
F_ON_ACT = {
    0: [True, True, True, True, True, False, False],
    1: [True, True, False, False, False, False, False],
}
PE_WARM_MMS = 28


def _ap(handle_ap, offset_elems, dims):
    base = handle_ap[tuple([slice(None)] * len(handle_ap.shape))]
    return bass.AP(tensor=base.tensor, offset=base.offset + offset_elems, ap=dims)


@with_exitstack
def _emit(ctx: ExitStack, tc: tile.TileContext):
    nc = tc.nc

    x_d = nc.dram_tensor("x", [ROWS_CORE, C], BF16, kind="ExternalInput")
    w1h_d = nc.dram_tensor("w1h", [128, 2, HID], F32, kind="ExternalInput")
    w1sh_d = nc.dram_tensor("w1sh", [128, 2, HID], F32, kind="ExternalInput")
    w2h_d = nc.dram_tensor("w2h", [HID, 2, 128], F32, kind="ExternalInput")
    b1c_d = nc.dram_tensor("b1c", [HID, 1], F32, kind="ExternalInput")
    b2t_d = nc.dram_tensor("b2t", [128, 2], F32, kind="ExternalInput")
    bands_d = nc.dram_tensor("bands", [H, 14, H], F32, kind="ExternalInput")
    identf_d = nc.dram_tensor("identf", [128, 128], F32, kind="ExternalInput")
    identb_d = nc.dram_tensor("identb", [128, 128], BF16, kind="ExternalInput")
    mask2_d = nc.dram_tensor("mask2", [128, 2], BF16, kind="ExternalInput")
    mask2t_d = nc.dram_tensor("mask2t", [2, 128], F32, kind="ExternalInput")
    convb_d = nc.dram_tensor("convb", [H, 1], F32, kind="ExternalInput")
    out_d = nc.dram_tensor("out", [ROWS_CORE, C], BF16, kind="ExternalOutput")

    xv = x_d[:, :].rearrange("(q p n) c -> q p n c", q=NPAIR, p=128)
    ov = out_d[:, :].rearrange("(q p n) c -> q p n c", q=NPAIR, p=128)

    constp = ctx.enter_context(tc.tile_pool(name="const", bufs=1))
    bigp = ctx.enter_context(tc.tile_pool(name="big", bufs=1))
    workp = ctx.enter_context(tc.tile_pool(name="work", bufs=1))
    psp1 = ctx.enter_context(tc.tile_pool(name="ps1", bufs=1, space="PSUM"))
    psp2 = ctx.enter_context(tc.tile_pool(name="ps2", bufs=2, space="PSUM"))

    # consts go on the scalar HWDGE ring so the sync ring starts streaming
    # x chunks immediately (per-lane depth-1 rings serialize completions)
    def const_load(name, shape, dtype, dram):
        t = constp.tile(shape, dtype, tag=name)
        nc.scalar.dma_start(t[tuple([slice(None)] * len(shape))], dram)
        return t

    # load order: earliest-needed consts first (scalar-ring DMAs serialize)
    mask2 = const_load("mask2", [128, 2], BF16, mask2_d[:, :])
    identb = const_load("identb", [128, 128], BF16, identb_d[:, :])
    b1c = const_load("b1c", [HID, 1], F32, b1c_d[:, :])
    b2t = const_load("b2t", [128, 2], F32, b2t_d[:, :])
    w1h = const_load("w1h", [128, 2, HID], F32, w1h_d[:, :, :])
    w1sh = const_load("w1sh", [128, 2, HID], F32, w1sh_d[:, :, :])
    w2h = const_load("w2h", [HID, 2, 128], F32, w2h_d[:, :, :])
    identf = const_load("identf", [128, 128], F32, identf_d[:, :])
    mask2t = const_load("mask2t", [2, 128], F32, mask2t_d[:, :])
    convb = const_load("convb", [H, 1], F32, convb_d[:, :])
    bands = const_load("bands", [H, 14, H], F32, bands_d[:, :, :])

    identfb, identbb = identf, identb
    w1hb, w1shb, w2hb, mask2tb = w1h, w1sh, w2h, mask2t

    bandsb = constp.tile([H, 14, H], F32R, tag="bandsb")

    # ACT sigmoid table preload (off critical path)
    warm = workp.tile([128, 8], F32, tag="warm")
    nc.vector.memset(warm[:, :], 0.0)
    nc.scalar.activation(out=warm[:, 0:8], in_=warm[:, 0:8], func=AF.Sigmoid,
                         bias=0.0, scale=1.0)

    # PE HAM warm-up: keep PE busy from t~8us so chsum matmuls run at 2.4GHz.
    # Source tile comes from a DVE memset, not a DMA, so this starts at once.
    warm_pe = workp.tile([128, 128], BF16, tag="warmpe")
    nc.vector.memset(warm_pe[:, :], 0.0)
    warm_ps = psp1.tile([128, 16], F32, tag="mlp")
    for _ in range(PE_WARM_MMS):
        nc.tensor.matmul(warm_ps[:, :], lhsT=warm_pe[:, :],
                         rhs=warm_pe[:, 0:16], start=True, stop=True)

    def load(q):
        """Issue pair q's DMA-in chunks."""
        X = bigp.tile([128, NBLK, C], BF16, tag=f"x{q}")
        for k in range(NCHUNK):
            nc.sync.dma_start(
                X[:, k * CHUNK:(k + 1) * CHUNK, :],
                xv[q, :, k * CHUNK:(k + 1) * CHUNK, :],
            )
        return X

    def stats(q, X):
        """channel-max folds on DVE + channel-sums on PE."""
        aw = workp.tile([128, CHUNK, C], BF16, tag=f"aw{q}")
        chs = psp2.tile([2, 512], F32, tag="chs")
        for k in range(NCHUNK):
            blk = X[:, k * CHUNK:(k + 1) * CHUNK, :]
            if k == 0:
                nc.vector.tensor_copy(aw[:], blk)
            else:
                nc.vector.tensor_max(aw[:], aw[:], blk)
        # channel sums: 24 pair-groups of N=512 + final single N=256
        for g in range(24):
            nc.tensor.matmul(
                chs[:, :], lhsT=mask2[:, :],
                rhs=X[:, 2 * g:2 * g + 2, :].rearrange("p a b -> p (a b)"),
                start=(g == 0), stop=False,
            )
        nc.tensor.matmul(chs[:, 0:256], lhsT=mask2[:, :], rhs=X[:, 48, :],
                         start=False, stop=True)
        # fold aw -> acc [128, 256]
        nc.vector.tensor_max(aw[:, 0:3, :], aw[:, 0:3, :], aw[:, 3:6, :])
        nc.vector.tensor_max(aw[:, 0, :], aw[:, 0, :], aw[:, 1, :])
        nc.vector.tensor_max(aw[:, 0, :], aw[:, 0, :], aw[:, 2, :])
        nc.vector.tensor_max(aw[:, 0, :], aw[:, 0, :], aw[:, 6, :])
        return aw[:, 0, :], chs

    def mlp(q, acc, chs):
        """channel gate from stats; returns cgb_bf [128, 256] bf16.

        Hop-minimized: copies/relu/sigmoid run on ACT (idle at this point,
        PSUM sources), the j-group sum and the avg+max add are folded into
        PE transpose accumulation groups. DVE only does the two max TRs."""
        statsT = workp.tile([128, 2, 2, 2], F32, tag=f"stats{q}")
        sum2 = workp.tile([2, 2, 256], F32, tag=f"sum{q}")
        nc.scalar.copy(sum2[:, :, :],
                       chs[:, :].rearrange("p (a b) -> p a b", a=2))
        mlp_ps = psp1.tile([128, 16], F32, tag="mlp")
        for h2 in range(2):
            tp_t = psp1.tile([128, 128], BF16, tag="tp")
            tp = tp_t[:, :]
            nc.tensor.transpose(tp, acc[:, h2 * 128:(h2 + 1) * 128],
                                identbb[:])
            nc.vector.tensor_reduce(
                out=statsT[:, h2, 1, :],
                in_=tp.rearrange("c (b p) -> c b p", b=2),
                axis=mybir.AxisListType.X, op=MU.max,
            )
            # avg stats: transpose the two 512-group halves, accumulated
            for j in range(2):
                nc.tensor.matmul(
                    mlp_ps[:, 2 * h2:2 * h2 + 2],
                    lhsT=sum2[:, j, h2 * 128:(h2 + 1) * 128],
                    rhs=identfb[0:2, 0:2],
                    is_transpose=True, start=(j == 0), stop=(j == 1),
                )
            nc.scalar.copy(
                statsT[:, h2, 0, :], mlp_ps[:, 2 * h2:2 * h2 + 2]
            )
        for stat in range(2):
            w1x = w1shb if stat == 0 else w1hb
            for h2 in range(2):
                nc.tensor.matmul(
                    mlp_ps[0:HID, 4 + 2 * stat:6 + 2 * stat],
                    lhsT=w1x[:, h2, :], rhs=statsT[:, h2, stat, :],
                    start=(h2 == 0), stop=(h2 == 1),
                )
        h_sb = workp.tile([HID, 2, 2], F32, tag=f"hsb{q}")
        nc.scalar.activation(
            out=h_sb[:],
            in_=mlp_ps[0:HID, 4:8].rearrange("p (s b) -> p s b", s=2),
            func=AF.Relu, bias=b1c[:], scale=1.0,
        )
        sigT = workp.tile([128, 2, 4], F32, tag=f"sig{q}")
        for h2 in range(2):
            cgp = mlp_ps[:, 8 + 4 * h2:12 + 4 * h2]
            nc.tensor.matmul(cgp, lhsT=w2hb[:, h2, :], rhs=h_sb[:, :, :],
                             start=True, stop=True)
            nc.scalar.activation(
                out=sigT[:, h2, :], in_=cgp, func=AF.Sigmoid,
                bias=b2t[:, h2:h2 + 1], scale=1.0,
            )
        cgr = workp.tile([2, 2, 128], F32, tag=f"cgr{q}")
        cgb_t = psp1.tile([128, C], F32, tag="cgb")
        cgb_ps = cgb_t[:, :]
        for h2 in range(2):
            tpr_t = psp1.tile([2, 128], F32, tag="tpr")
            tpr = tpr_t[:, :]
            # avg + max sigmoid outputs added via transpose accumulation
            for part in range(2):
                nc.tensor.matmul(
                    tpr, lhsT=sigT[:, h2, 2 * part:2 * part + 2],
                    rhs=identfb[:, :],
                    is_transpose=True, start=(part == 0), stop=(part == 1),
                )
            nc.scalar.copy(cgr[:, h2, :], tpr)
            nc.tensor.matmul(
                cgb_ps[:, h2 * 128:(h2 + 1) * 128],
                lhsT=mask2tb[:], rhs=cgr[:, h2, :],
                start=True, stop=True,
            )
        cgb = workp.tile([128, C], BF16, tag=f"cgb{q}")
        nc.scalar.copy(cgb[:], cgb_ps[:])
        return cgb

    def gate_mult(q, X, cgb):
        """xg = x * cg in place, chunked (bf16 TT 2x with broadcast AP)."""
        cgb_rep = bass.AP(tensor=cgb.tensor, offset=cgb.offset,
                          ap=[cgb.ap[0], [0, CHUNK], cgb.ap[1]])
        for k in range(NCHUNK):
            blk = X[:, k * CHUNK:(k + 1) * CHUNK, :]
            nc.vector.tensor_tensor(out=blk, in0=blk, in1=cgb_rep, op=MU.mult)

    def spatial_stats(q, X):
        """smax/savg fold trees at pair granularity.
        ssb layout [128, 2(stat), 7, 8]: 49 values as 7 stride-8 rows of 7 so
        the SBUF->SBUF shuffle DMA keeps a non-collapsible inner dim of 7."""
        fb = workp.tile([128, NBLK, 128], BF16, tag=f"fb{q}")
        ssb = workp.tile([128, 2, CHUNK, 8], F32, tag=f"ssb{q}")
        for stat, op in ((1, MU.max), (0, MU.add)):
            nc.vector.tensor_tensor(out=fb[:, :, :], in0=X[:, :, 0:128],
                                    in1=X[:, :, 128:256], op=op)
            nc.vector.tensor_tensor(out=fb[:, :, 0:64], in0=fb[:, :, 0:64],
                                    in1=fb[:, :, 64:128], op=op)
            nc.vector.tensor_tensor(out=fb[:, :, 0:32], in0=fb[:, :, 0:32],
                                    in1=fb[:, :, 32:64], op=op)
            nc.vector.tensor_reduce(out=ssb[:, stat, :, 0:7],
                                    in_=fb[:, :, 0:32],
                                    axis=mybir.AxisListType.X, op=op)
        return ssb

    def conv(q, ssb):
        """7x7x2->1 conv: direct SBUF->SBUF reshuffles (no DRAM bounce),
        f32r band matmuls, sigmoid, direct gather back."""
        s_sb = workp.tile([H, 2, 2, 62], F32, tag=f"ssb2{q}")
        nc.vector.memset(s_sb[:], 0.0)
        # (p', n) walk == (h, w) walk per (ic, b): flat s = 49 p' + n = 56 h + w
        for ic in range(2):
            for b in range(2):
                nc.sync.dma_start(
                    s_sb[0:H, ic, b, 3:3 + W],
                    ssb[64 * b:64 * (b + 1), ic, :, 0:7],
                )
        s_sb2 = workp.tile([H, 2, 2, 62], F32R, tag=f"ssb3{q}")
        nc.vector.tensor_copy(s_sb2[:], s_sb[:])
        conv_ps = psp2.tile([H, 2, W], F32, tag="conv")
        for ic in range(2):
            for dw in range(7):
                j = ic * 7 + dw
                nc.tensor.matmul(
                    conv_ps[:], lhsT=bandsb[:, j, :],
                    rhs=s_sb2[:, ic, :, dw:dw + W],
                    start=(j == 0), stop=(j == 13),
                )
        sg_hw = workp.tile([H, 2, W], F32, tag=f"sghw{q}")
        nc.scalar.activation(out=sg_hw[:], in_=conv_ps[:], func=AF.Sigmoid,
                             bias=convb[:], scale=1.0)
        sg = workp.tile([128, CHUNK, 8], F32, tag=f"sg{q}")
        for b in range(2):
            nc.sync.dma_start(
                sg[64 * b:64 * (b + 1), :, 0:7],
                sg_hw[0:H, b, :],
            )
        return sg

    def finalize(q, X, sg, chunks, on_act):
        """out = xg * sg for the selected chunks, then DMA-out each chunk."""
        for k in chunks:
            for n in range(k * CHUNK, (k + 1) * CHUNK):
                sgn = sg[:, n // 7, (n % 7):(n % 7) + 1]
                if on_act:
                    nc.scalar.mul(X[:, n, :], X[:, n, :], mul=sgn)
                else:
                    nc.vector.tensor_scalar_mul(X[:, n, :], X[:, n, :], sgn)
            nc.sync.dma_start(
                ov[q, :, k * CHUNK:(k + 1) * CHUNK, :],
                X[:, k * CHUNK:(k + 1) * CHUNK, :],
            )

    def act_chunks(q):
        return [k for k in range(NCHUNK) if F_ON_ACT[q][k]]

    def dve_chunks(q):
        return [k for k in range(NCHUNK) if not F_ON_ACT[q][k]]

    # pipeline-ordered emission. Both loads are issued first (SP ring);
    # pair 1's DVE stats come AFTER C0 so the late-arriving X1 chunks don't
    # block C0 in the in-order DVE queue. F's DVE chunks come last.
    X0 = load(0)
    X1 = load(1)
    acc0, chs0 = stats(0, X0)
    cgb0 = mlp(0, acc0, chs0)
    acc1, chs1 = stats(1, X1)
    cgb1 = mlp(1, acc1, chs1)
    gate_mult(0, X0, cgb0)
    # f32r producer for the conv band matmuls, emitted here so the DVE copy
    # doesn't head-block the queue waiting for the bands const DMA
    nc.vector.tensor_copy(bandsb[:, :, :], bands[:, :, :])
    ssb0 = spatial_stats(0, X0)
    sg0 = conv(0, ssb0)
    finalize(0, X0, sg0, act_chunks(0), on_act=True)
    gate_mult(1, X1, cgb1)
    ssb1 = spatial_stats(1, X1)
    sg1 = conv(1, ssb1)
    finalize(0, X0, sg0, dve_chunks(0), on_act=False)
    finalize(1, X1, sg1, dve_chunks(1), on_act=False)
    finalize(1, X1, sg1, act_chunks(1), on_act=True)


def _split_evsem_clears(nc):
    """Walrus rejects EVENT_SEMAPHORE_RANGE_CLEAR over wide sem ranges;
    split into clears of <=3 sems."""
    for f in nc.m.functions:
        for blk in f.blocks:
            il = blk.instructions
            for i in range(len(il)):
                inst = il[i]
                if type(inst).__name__ != 'InstISA':
                    continue
                d = inst.ant_dict
                if d is None or 'range_first' not in d or 'range_last' not in d:
                    continue
                first, last = d['range_first'], d['range_last']
                if last - first + 1 <= 3:
                    continue
                si = inst.sync_info
                import copy
                reps = []
                a = first
                while a <= last:
                    b = min(a + 2, last)
                    cl = copy.deepcopy(inst)
                    cl.name = f"I-ws{nc.next_id()}"
                    cd = cl.ant_dict
                    cd['range_first'] = a
                    cd['range_last'] = b
                    reps.append(cl)
                    a = b + 1
                reps[0].sync_info = si
                il[i] = reps[0]
                for j, r in enumerate(reps[1:]):
                    il.insert(i + 1 + j, r)
                break


def _split_waits(nc):
    """Walrus accepts at most ONE sync wait per engine instruction; split
    surplus waits onto injected drain carriers (same engine, order kept)."""
    import copy

    proto = {}
    for f in nc.m.functions:
        for blk in f.blocks:
            for inst in blk.instructions:
                if type(inst).__name__ == 'InstDrain' and inst.engine not in proto:
                    proto[inst.engine] = inst
    for f in nc.m.functions:
        for blk in f.blocks:
            il = blk.instructions
            i = 0
            while i < len(il):
                inst = il[i]
                si = inst.sync_info
                if si is None or len(si.on_wait) <= 1:
                    i += 1
                    continue
                waits = list(si.on_wait)
                eng = inst.engine
                for w in waits[:-1]:
                    nop = copy.deepcopy(proto[eng])
                    nop.name = f"I-ws{nc.next_id()}"
                    nop.sync_info = type(si)(on_wait=[w], on_update=[])
                    il.insert(i, nop)
                    i += 1
                inst.sync_info = type(si)(
                    on_wait=[waits[-1]], on_update=list(si.on_update)
                )
                i += 1


_NC = {}


def _get_nc(split=True):
    if split not in _NC:
        nc = bass.Bass()
        with tile.TileContext(nc) as tc:
            _emit(tc)
        if split:
            _split_waits(nc)
            _split_evsem_clears(nc)
        _NC[split] = nc
    return _NC[split]


def _host_inputs(w1, b1, w2, b2, conv_w, conv_b):
    w1 = np.asarray(w1, np.float32)
    w2 = np.asarray(w2, np.float32)
    w1h = np.ascontiguousarray(w1.reshape(2, 128, HID).transpose(1, 0, 2))
    w1sh = np.ascontiguousarray(w1h / float(SP))
    w2h = np.ascontiguousarray(np.asarray(w2, np.float32).reshape(HID, 2, 128))
    b1c = np.ascontiguousarray(np.asarray(b1, np.float32).reshape(HID, 1))
    b2t = np.ascontiguousarray(np.asarray(b2, np.float32).reshape(2, 128).T)
    cw = np.asarray(conv_w, np.float32).reshape(7, 7, 2)
    bands = np.zeros((H, 14, H), np.float32)
    for ic in range(2):
        for dw in range(7):
            for dh in range(7):
                d = dh - 3
                v = cw[dh, dw, ic]
                if ic == 0:
                    v = v / float(C)  # fold 1/C of s_avg into avg bands
                if d >= 0:
                    idx = np.arange(0, H - d)
                    bands[idx + d, ic * 7 + dw, idx] = v
                else:
                    idx = np.arange(-d, H)
                    bands[idx + d, ic * 7 + dw, idx] = v
    identf = np.eye(128, dtype=np.float32)
    identb = np.eye(128, dtype=np.float32).astype(ml_dtypes.bfloat16)
    mask2 = np.zeros((128, 2), np.float32)
    mask2[0:64, 0] = 1.0
    mask2[64:128, 1] = 1.0
    mask2b = mask2.astype(ml_dtypes.bfloat16)
    mask2t = np.ascontiguousarray(mask2.T)
    convb = np.full((H, 1), np.asarray(conv_b, np.float32).reshape(-1)[0],
                    np.float32)
    return dict(w1h=w1h, w1sh=w1sh, w2h=w2h, b1c=b1c, b2t=b2t,
                bands=bands, identf=identf, identb=identb, mask2=mask2b,
                mask2t=mask2t, convb=convb)


def kernel(x, w1, b1, w2, b2, conv_w, conv_b, _trace=False):
    from concourse.bass_utils import run_bass_kernel_spmd

    nc = _get_nc()
    consts = _host_inputs(w1, b1, w2, b2, conv_w, conv_b)
    xb = np.asarray(x, np.float32).astype(ml_dtypes.bfloat16)
    xs = np.ascontiguousarray(xb).reshape(8, ROWS_CORE, C)
    in_maps = [dict(consts, x=xs[i]) for i in range(N_CORES)]
    res = run_bass_kernel_spmd(nc, in_maps, core_ids=list(range(N_CORES)),
                               trace=_trace)
    out = np.stack([np.asarray(r["out"]) for r in res.results])
    out = out.astype(np.float32).reshape(32, H, W, C)
    if _trace:
        kernel.last_results = res
    return out
